# revision 26
# baseline (speedup 1.0000x reference)
"""BatchedGraphSAGEDynamicRangeMean kernel for 8 Trainium2 NeuronCores.

Sharding: data-parallel over batch b -- core c computes graph c entirely
(N=4096 nodes, D=256), BN statistics all-reduced across the 8 cores.

Per-core pipeline (32 row-blocks of 128 nodes):
  setup: stream x in (8 chunked DMAs), row sum-squares via fused
    square+row-reduce, xn = x*rsqrt(ssq), Xn^T via PE transposes (bf16
    identity moving operand -> 1 cycle/row), F = x @ (0.5*Wn)^T
    precomputed per block (norm un-scale folded into the PSUM->SBUF copy).
  main per block z: banded cosine sims as f32r matmuls (1 cycle/row,
    exact fp32 in this toolchain) + window-validity mask folded in as a
    rank-8 PSUM-accumulated matmul; hardware max8 -> v3 = 3rd largest;
    C = (S >= v3) - selfdiag (bf16); PE-transpose C; h2 = C^T-chunks @ F
    directly (neighbour mean + Wn fused); g1 = Xn @ WxT; row l2norm via
    fused square+reduce on both halves; relu+scale copies into bf16 h;
    BN partial sums via ones-vector matmuls accumulated in PSUM.
  AllReduce(2x512) -> channel scale/bias -> BN apply: bf16 multiply on
    DVE, bias-add folded into a PE identity-matmul into PSUM, output
    DMA'd straight from PSUM.
"""

import threading
import numpy as np

B, N, D, DOUT = 8, 4096, 256, 256
P = 128
NB = N // P            # 32 blocks
CAND = 3 * P           # 384 candidate columns per block
NCORES = 8
CH = 2 * DOUT          # 512 output channels
NEG = -1.0e30
EPS_BN = 1e-5

_cache = {}
_lock = threading.Lock()


def _build(single=False, nobias=True):
    import concourse.bass as bass
    from concourse import bacc
    import concourse.mybir as mybir
    import concourse.tile as tile
    from concourse.masks import make_identity

    f32 = mybir.dt.float32
    f32r = mybir.dt.float32r
    bf16 = mybir.dt.bfloat16
    f16 = mybir.dt.float16
    AF = mybir.ActivationFunctionType
    OP = mybir.AluOpType

    nc = bacc.Bacc("TRN2", target_bir_lowering=False)
    x_in = nc.declare_dram_parameter("xb", [N, D], f32, isOutput=False)
    wxT_in = nc.declare_dram_parameter("wxT", [D, DOUT], f16, isOutput=False)
    wnTh_in = nc.declare_dram_parameter("wnTh", [D, DOUT], f16, isOutput=False)
    gamma_in = nc.declare_dram_parameter("gamma", [1, CH], f32, isOutput=False)
    beta_in = nc.declare_dram_parameter("beta", [1, CH], f32, isOutput=False)
    wm_in = nc.declare_dram_parameter("winmask", [8, 3, CAND], f16, isOutput=False)
    ind_in = nc.declare_dram_parameter("indi", [8, P], f16, isOutput=False)
    if not nobias:
        bx_in = nc.declare_dram_parameter("bx", [1, DOUT], f32, isOutput=False)
        bn_in = nc.declare_dram_parameter("bn", [1, DOUT], f32, isOutput=False)
    out_ext = nc.declare_dram_parameter("out", [N, CH], f32, isOutput=True)

    with tile.TileContext(nc) as tc:
        with (
            tc.tile_pool(name="persist", bufs=1) as pp,
            tc.tile_pool(name="work", bufs=2) as wp,
            tc.tile_pool(name="psS", bufs=2, space="PSUM") as psS,
            tc.tile_pool(name="psG", bufs=2, space="PSUM") as psG,
            tc.tile_pool(name="psM", bufs=3, space="PSUM") as psM,
            tc.tile_pool(name="ps_st", bufs=1, space="PSUM") as ps_st,
            tc.tile_pool(name="dram", bufs=1, space="DRAM") as dp,
        ):
            # ---------------- persistent tensors ----------------
            xstage = pp.tile([P, NB, D], f32)         # raw x, block-major
            xpT = pp.tile([P, 4, N + 2 * P], f16)     # Xn^T hi/lo fp16 pair
            Ft = pp.tile([P, NB, DOUT], bf16)         # F = x @ (0.5 Wn)^T
            hsb = pp.tile([P, NB, CH], bf16)          # h (pre-BN), bf16
            identity = pp.tile([P, P], f32)
            identB = pp.tile([P, P], bf16)
            masks = pp.tile([8, 3, CAND], f16)
            indi = pp.tile([8, P], f16)
            ones1_r = pp.tile([1, P], f32r)
            diagS = pp.tile([P, CAND], bf16)
            wx = pp.tile([P, 2, DOUT], f16)
            wn = pp.tile([P, 2, DOUT], f16)
            ssq = pp.tile([P, NB], f32)
            inv = pp.tile([P, NB], f32)
            norms = pp.tile([P, NB], f32)
            gam4 = pp.tile([4, P], f32)
            bet4 = pp.tile([4, P], f32)
            muq = pp.tile([4, P], f32)
            e2q = pp.tile([4, P], f32)
            v4 = pp.tile([4, P], f32)
            s4 = pp.tile([4, P], f32)
            b4 = pp.tile([4, P], f32)
            srow = pp.tile([1, CH], f32r)
            brow = pp.tile([1, CH], f32r)
            sbc = pp.tile([P, CH], bf16)
            bbc = pp.tile([P, CH], f32)
            st_sb = pp.tile([33, CH], f32)
            eps_t = pp.tile([4, 1], f32)
            ones_col = pp.tile([P, 1], bf16)
            if not nobias:
                bx_row = pp.tile([1, DOUT], f32r)
                bn_row = pp.tile([1, DOUT], f32r)
                invT2 = pp.tile([1, NB, P], f32r)
                invT = pp.tile([NB, P], f32)

            make_identity(nc, identity)
            nc.vector.tensor_copy(identB, identity)
            nc.gpsimd.memset(eps_t, EPS_BN)
            ones_colf = pp.tile([P, 1], f32)
            nc.gpsimd.memset(ones_colf, 1.0)
            nc.vector.tensor_copy(ones_col, ones_colf)
            ones_rowf = pp.tile([1, P], f32)
            nc.gpsimd.memset(ones_rowf, 1.0)
            nc.vector.tensor_copy(ones1_r, ones_rowf)
            zscr = wp.tile([P, P], f32, tag="zpad")
            nc.gpsimd.memset(zscr, 0.0)
            for c in range(4):
                nc.vector.tensor_copy(xpT[:, c, 0:P], zscr)
                nc.vector.tensor_copy(xpT[:, c, N + P:N + 2 * P], zscr)
            nc.gpsimd.memset(diagS, 0.0)
            nc.vector.tensor_copy(diagS[:, P:2 * P], identB)

            nc.sync.dma_start(masks, wm_in[:, :, :])
            nc.sync.dma_start(indi, ind_in[:, :])
            for c in range(2):
                nc.sync.dma_start(wx[:, c, :], wxT_in[P * c:P * (c + 1), :])
                nc.sync.dma_start(wn[:, c, :], wnTh_in[P * c:P * (c + 1), :])
            nc.sync.dma_start(
                gam4, gamma_in[:, :].rearrange("a (q c) -> (a q) c", c=P))
            nc.sync.dma_start(
                bet4, beta_in[:, :].rearrange("a (q c) -> (a q) c", c=P))
            if not nobias:
                nc.sync.dma_start(bx_row, bx_in[:, :].bitcast(f32r))
                nc.sync.dma_start(bn_row, bn_in[:, :].bitcast(f32r))

            # x in 8 chunked DMAs of 4 blocks each
            for j in range(8):
                nc.sync.dma_start(
                    xstage[:, 4 * j:4 * (j + 1), :],
                    x_in[512 * j:512 * (j + 1), :]
                    .rearrange("(z p) d -> p z d", p=P))

            # ---------------- setup A: row sum-squares ----------------
            for z in range(NB):
                xsq = wp.tile([P, D], bf16, tag="xsq")
                nc.vector.scalar_tensor_tensor(
                    out=xsq, in0=xstage[:, z, :], scalar=1.0,
                    in1=xstage[:, z, :],
                    op0=OP.mult, op1=OP.mult, accum_out=ssq[:, z:z + 1])
                if z % 4 == 3:
                    j = z // 4
                    sl = slice(4 * j, 4 * j + 4)
                    nc.scalar.activation(out=norms[:, sl], in_=ssq[:, sl],
                                         func=AF.Sqrt)
                    nc.vector.reciprocal(out=inv[:, sl], in_=norms[:, sl])
            if not nobias:
                trv_ps = ps_st.tile([NB, P], f32, tag="sth")
                nc.tensor.transpose(trv_ps, inv[:, 0:NB], identity)
                nc.vector.tensor_copy(invT, trv_ps)
                invT_d = dp.tile([NB, P], f32)
                nc.sync.dma_start(invT_d, invT)
                nc.sync.dma_start(
                    invT2,
                    invT_d[:, :].rearrange("a b -> (a b)")[None, :].bitcast(f32r))

            st_h = ps_st.tile([33, CH], f32, tag="sth")

            # ---------------- setup B + main loop, interleaved ----------------
            def setup_B(z):
                # xn = x * inv
                xn = wp.tile([P, D], f32, tag="xn")
                nc.gpsimd.tensor_scalar(out=xn,
                                        in0=xstage[:, z, :],
                                        scalar1=inv[:, z:z + 1], scalar2=None,
                                        op0=OP.mult)
                tr_ps = psM.tile([P, 2, P], f32, tag="m")
                for c in range(2):
                    nc.tensor.transpose(tr_ps[:, c, :], xn[:, P * c:P * (c + 1)],
                                        identity)
                ccol = P * (z + 1)
                # hi = fp16(xnT); lo = fp16(xnT - hi)
                nc.scalar.activation(out=xpT[:, 0:2, ccol:ccol + P],
                                     in_=tr_ps[:, :, :], func=AF.Copy)
                nc.vector.scalar_tensor_tensor(
                    out=xpT[:, 2:4, ccol:ccol + P], in0=tr_ps[:, :, :],
                    scalar=1.0, in1=xpT[:, 0:2, ccol:ccol + P],
                    op0=OP.mult, op1=OP.subtract)
                # F[z] = x @ (0.5 Wn)^T  (+ 0.5*bn if biased)
                fn_ps = psG.tile([P, DOUT], f32, tag="g1")
                for c in range(2):
                    nc.tensor.matmul(fn_ps, xpT[:, c, ccol:ccol + P],
                                     wn[:, c, :], start=(c == 0),
                                     stop=(c == 1 and nobias))
                if not nobias:
                    nc.tensor.matmul(fn_ps, invT2[:, z, :], bn_row,
                                     start=False, stop=True)
                nc.scalar.activation(out=Ft[:, z, :], in_=fn_ps,
                                     func=AF.Copy, scale=norms[:, z:z + 1])

            def main_M(z):
                cstart = P * (z + 1)
                # sims + window mask, single PSUM accumulation group
                sim_ps = psS.tile([P, CAND], f32, tag="sim")
                first = True
                for c in range(2):
                    for (qs, cs) in ((0, 0), (0, 2), (2, 0)):
                        nc.tensor.matmul(sim_ps,
                                         xpT[:, qs + c, cstart:cstart + P],
                                         xpT[:, cs + c, P * z:P * z + CAND],
                                         start=first, stop=False)
                        first = False
                v = 0 if z == 0 else (2 if z == NB - 1 else 1)
                nc.tensor.matmul(sim_ps, indi, masks[:, v, :],
                                 start=False, stop=True)

                top8 = wp.tile([P, 8], f32, tag="top8")
                nc.vector.max(out=top8, in_=sim_ps)
                Cm = wp.tile([P, CAND], bf16, tag="cm")
                nc.vector.scalar_tensor_tensor(out=Cm, in0=sim_ps,
                                               scalar=top8[:, 2:3], in1=diagS,
                                               op0=OP.is_ge, op1=OP.subtract)
                ct_ps = psM.tile([P, 3, P], bf16, tag="m")
                for k in range(3):
                    nc.tensor.transpose(ct_ps[:, k, :], Cm[:, P * k:P * (k + 1)],
                                        identB)
                ct_sb = wp.tile([P, 3, P], bf16, tag="ct_sb")
                nc.vector.tensor_copy(ct_sb, ct_ps)

                # h2 = C^T-chunks @ F rows (mean of 2 neighbours, Wn fused)
                h2_ps = psM.tile([P, DOUT], f32, tag="m")
                for k in range(3):
                    zk = min(max(z - 1 + k, 0), NB - 1)
                    nc.tensor.matmul(h2_ps, ct_sb[:, k, :], Ft[:, zk, :],
                                     start=(k == 0), stop=(k == 2))
                # g1 = Xn @ WxT (+ inv*bx if biased)
                g1_ps = psG.tile([P, DOUT], f32, tag="g1")
                for c in range(2):
                    nc.tensor.matmul(g1_ps, xpT[:, c, cstart:cstart + P],
                                     wx[:, c, :], start=(c == 0),
                                     stop=(c == 1 and nobias))
                if not nobias:
                    nc.tensor.matmul(g1_ps, invT2[:, z, :], bx_row,
                                     start=False, stop=True)

                # row l2 norms of [h1, h2]
                scrA = wp.tile([P, DOUT], bf16, tag="scrA")
                sA = wp.tile([P, 1], f32, tag="sA")
                nc.scalar.activation(out=scrA, in_=g1_ps, func=AF.Square,
                                     accum_out=sA)
                scrB = wp.tile([P, DOUT], bf16, tag="scrB")
                sB = wp.tile([P, 1], f32, tag="sB")
                nc.scalar.activation(out=scrB, in_=h2_ps, func=AF.Square,
                                     accum_out=sB)
                tot = wp.tile([P, 1], f32, tag="tot")
                nc.vector.scalar_tensor_tensor(out=tot, in0=sA,
                                               scalar=ssq[:, z:z + 1], in1=sB,
                                               op0=OP.mult, op1=OP.add)
                hno = wp.tile([P, 1], f32, tag="hno")
                nc.scalar.activation(out=hno, in_=tot, func=AF.Sqrt)
                rinv = wp.tile([P, 1], f32, tag="rinv")
                nc.vector.reciprocal(out=rinv, in_=hno)
                s1 = wp.tile([P, 1], f32, tag="s1")
                nc.vector.tensor_mul(s1, norms[:, z:z + 1], rinv)
                nc.scalar.activation(out=hsb[:, z, 0:DOUT], in_=g1_ps,
                                     func=AF.Relu, scale=s1)
                nc.vector.tensor_scalar(out=hsb[:, z, DOUT:CH], in0=h2_ps,
                                        scalar1=rinv, scalar2=0.0,
                                        op0=OP.mult, op1=OP.max)

                # BN partial sums, accumulated in PSUM across blocks
                hsq = wp.tile([P, CH], bf16, tag="hsq")
                nc.gpsimd.tensor_mul(hsq, hsb[:, z, :], hsb[:, z, :])
                nc.tensor.matmul(st_h[0:1, :], ones_col, hsb[:, z, :],
                                 start=(z == 0), stop=(z == NB - 1))
                nc.tensor.matmul(st_h[32:33, :], ones_col, hsq,
                                 start=(z == 0), stop=(z == NB - 1))

            for z in range(NB):
                setup_B(z)
                if z >= 1:
                    main_M(z - 1)
            main_M(NB - 1)

            # ---------------- BN stats all-reduce ----------------
            st_in_d = dp.tile([2, CH], f32)
            st_out_d = dp.tile([2, CH], f32)
            nc.vector.tensor_copy(st_sb[0:1, :], st_h[0:1, :])
            nc.vector.tensor_copy(st_sb[32:33, :], st_h[32:33, :])
            nc.sync.dma_start(st_in_d[0:1, :], st_sb[0:1, :])
            nc.sync.dma_start(st_in_d[1:2, :], st_sb[32:33, :])
            if single:
                nc.sync.dma_start(st_out_d, st_in_d[:, :])
            else:
                nc.gpsimd.collective_compute(
                    "AllReduce", mybir.AluOpType.add,
                    replica_groups=[list(range(NCORES))],
                    ins=[st_in_d[:].opt()],
                    outs=[st_out_d[:].opt()],
                )
            sc = 1.0 / float(B * N)
            nc.sync.dma_start(
                muq, st_out_d[0:1, :].rearrange("a (q c) -> (a q) c", c=P))
            nc.sync.dma_start(
                e2q, st_out_d[1:2, :].rearrange("a (q c) -> (a q) c", c=P))
            nc.vector.tensor_scalar_mul(muq, muq, sc)          # mu
            nc.vector.tensor_scalar_mul(e2q, e2q, sc)          # E[h^2]
            nc.vector.tensor_mul(v4, muq, muq)                 # mu^2
            nc.vector.tensor_sub(v4, e2q, v4)                  # var
            nc.scalar.activation(out=v4, in_=v4, func=AF.Sqrt,
                                 bias=eps_t)
            nc.vector.reciprocal(out=v4, in_=v4)               # rstd
            nc.vector.tensor_mul(s4, v4, gam4)                 # s = gamma*rstd
            nc.vector.tensor_mul(b4, muq, s4)
            nc.vector.tensor_sub(b4, bet4, b4)                 # b = beta - mu*s
            # bounce s/b through DRAM to get [1, CH] row layout
            sb_d = dp.tile([2, 4, P], f32)
            nc.sync.dma_start(sb_d[0], s4)
            nc.sync.dma_start(sb_d[1], b4)
            nc.sync.dma_start(
                srow, sb_d[0].rearrange("q c -> (q c)")[None, :].bitcast(f32r))
            nc.sync.dma_start(
                brow, sb_d[1].rearrange("q c -> (q c)")[None, :].bitcast(f32r))
            bc_ps = psG.tile([P, CH], f32, tag="g1")
            nc.tensor.matmul(bc_ps, ones1_r, srow, start=True, stop=True)
            nc.vector.tensor_copy(sbc, bc_ps)
            bc_ps2 = psG.tile([P, CH], f32, tag="g1")
            nc.tensor.matmul(bc_ps2, ones1_r, brow, start=True, stop=True)
            nc.vector.tensor_copy(bbc, bc_ps2)

            # ---------------- BN apply + writeback ----------------
            for z in range(NB):
                tmul = wp.tile([P, CH], bf16, tag="tmul")
                if z % 4 == 3:
                    nc.gpsimd.tensor_mul(tmul, hsb[:, z, :], sbc)
                else:
                    nc.vector.tensor_mul(tmul, hsb[:, z, :], sbc)
                out_t = wp.tile([P, CH], f32, tag="out_t")
                if z % 3 == 2:
                    nc.gpsimd.tensor_add(out_t, tmul, bbc)
                else:
                    nc.vector.tensor_add(out_t, tmul, bbc)
                nc.sync.dma_start(out_ext[P * z:P * (z + 1), :], out_t)

    return _finish(nc)


def _finish(nc):
    nc.finalize()
    return nc


def _get_nc(nobias=True):
    with _lock:
        key = ("nc", nobias)
        if key not in _cache:
            _cache[key] = _build(nobias=nobias)
        return _cache[key]


def _host_params():
    wm = np.full((8, 3, CAND), -60000.0, dtype=np.float16)
    for g in range(8):
        lo, hi = 16 * g, 16 * g + 272
        wm[g, 1, lo:hi] = 0.0                      # interior
        wm[g, 0, max(lo, 128):hi] = 0.0            # z == 0 (left edge)
        wm[g, 2, lo:min(hi, 256)] = 0.0            # z == NB-1 (right edge)
    ind = np.zeros((8, P), dtype=np.float16)
    for g in range(8):
        ind[g, 16 * g:16 * g + 16] = 1.0
    return wm, ind


def _run(inputs, trace=False, trace_kwargs=None):
    from concourse.bass_utils import run_bass_kernel_spmd

    x = np.ascontiguousarray(np.asarray(inputs["x"], dtype=np.float32))
    Wx_w = np.asarray(inputs["Wx_w"], dtype=np.float32)
    Wx_b = np.asarray(inputs["Wx_b"], dtype=np.float32)
    Wn_w = np.asarray(inputs["Wn_w"], dtype=np.float32)
    Wn_b = np.asarray(inputs["Wn_b"], dtype=np.float32)
    gamma = np.asarray(inputs["gamma"], dtype=np.float32)
    beta = np.asarray(inputs["beta"], dtype=np.float32)
    assert x.shape == (B, N, D), x.shape
    assert int(inputs["p"]) == 16 and int(inputs["t"]) == 8

    nobias = bool(np.all(Wx_b == 0.0) and np.all(Wn_b == 0.0))
    wxT = np.ascontiguousarray(Wx_w.T.astype(np.float16))
    wnTh = np.ascontiguousarray((0.5 * Wn_w).T.astype(np.float16))
    wm, ind = _host_params()
    shared = {
        "wxT": wxT, "wnTh": wnTh, "winmask": wm, "indi": ind,
        "gamma": gamma.reshape(1, CH), "beta": beta.reshape(1, CH),
    }
    if not nobias:
        shared["bx"] = Wx_b.reshape(1, DOUT)
        shared["bn"] = (0.5 * Wn_b).reshape(1, DOUT)
    in_maps = [{"xb": np.ascontiguousarray(x[c]), **shared} for c in range(NCORES)]

    nc = _get_nc(nobias=nobias)
    kw = {}
    if trace:
        kw = dict(trace=True, trace_kwargs=trace_kwargs or {})
    res = run_bass_kernel_spmd(nc, in_maps, core_ids=list(range(NCORES)), **kw)
    out = np.stack([res.results[c]["out"] for c in range(NCORES)], axis=0)
    return out.astype(np.float32), res


def kernel(**inputs):
    out, _ = _run(inputs)
    return out


# revision 32
# speedup vs baseline: 1.4605x; 1.4605x over previous
"""BatchedGraphSAGEDynamicRangeMean kernel for 8 Trainium2 NeuronCores.

Sharding: data-parallel over batch b -- core c computes graph c entirely
(N=4096 nodes, D=256), BN statistics all-reduced across the 8 cores.

Per-core pipeline (32 row-blocks of 128 nodes), software-pipelined with
staged emission so every engine always has ready work:
  setup B(z): xh/xl = fp16 hi/lo split of xn = x*rsqrt(ssq) (exact fp32
    pair), 4 PE transposes -> XnT hi/lo; [G|F] = x @ [Wx^T | 0.5*Wn^T]
    per block (norm un-scale folded into the PSUM->SBUF copy, bf16).
  M1(z): banded cosine sims = 6 fp16 matmuls (hh+hl+lh, fp32-exact) +
    window-validity mask folded in as a rank-8 fp16 matmul accumulated
    into the same PSUM group; hardware max8 -> v3 = 3rd-largest.
  M2(z): C = (S >= v3) - selfdiag (bf16, exact); 3 PE transposes.
  M3(z): h2 = C^T-chunks @ F rows (neighbour mean + Wn fused); store
    UN-normalized relu halves hr = [relu(G), relu(h2)] (bf16) -- the row
    1/l2norm scale is deferred, so nothing on this path waits for it.
  M4(z): row sum-squares of G / h2 (fused square+row-reduce); hq = hr^2;
    BN partial sums via matmuls whose lhsT column carries rinv / rinv^2.
  Every 4 blocks: batched scale chain rinv = 1/sqrt(sumsq) on [P,4].
  AllReduce(2x512) -> channel scale/bias; apply = one fused
  (hr*rinv)*sbc multiply (per-row x per-channel) + bias add + DMA.
"""

import threading
import numpy as np

B, N, D, DOUT = 8, 4096, 256, 256
P = 128
NB = N // P            # 32 blocks
CAND = 3 * P           # 384 candidate columns per block
NCORES = 8
CH = 2 * DOUT          # 512 output channels
EPS_BN = 1e-5

_cache = {}
_lock = threading.Lock()


def _build(single=False, nobias=True):
    import concourse.bass as bass
    from concourse import bacc
    import concourse.mybir as mybir
    import concourse.tile as tile
    from concourse.masks import make_identity

    f32 = mybir.dt.float32
    f32r = mybir.dt.float32r
    bf16 = mybir.dt.bfloat16
    f16 = mybir.dt.float16
    AF = mybir.ActivationFunctionType
    OP = mybir.AluOpType

    nc = bacc.Bacc("TRN2", target_bir_lowering=False)
    x_in = nc.declare_dram_parameter("xb", [N, D], f32, isOutput=False)
    wfg_in = nc.declare_dram_parameter("wfg", [D, CH], f16, isOutput=False)
    gamma_in = nc.declare_dram_parameter("gamma", [1, CH], f32, isOutput=False)
    beta_in = nc.declare_dram_parameter("beta", [1, CH], f32, isOutput=False)
    wm_in = nc.declare_dram_parameter("winmask", [8, 3, CAND], f16, isOutput=False)
    ind_in = nc.declare_dram_parameter("indi", [8, P], f16, isOutput=False)
    if not nobias:
        bxn_in = nc.declare_dram_parameter("bxn", [1, CH], f32, isOutput=False)
    out_ext = nc.declare_dram_parameter("out", [N, CH], f32, isOutput=True)

    with tile.TileContext(nc) as tc:
        with (
            tc.tile_pool(name="persist", bufs=1) as pp,
            tc.tile_pool(name="work", bufs=3) as wp,
            tc.tile_pool(name="psS", bufs=2, space="PSUM") as psS,
            tc.tile_pool(name="psF", bufs=2, space="PSUM") as psF,
            tc.tile_pool(name="psM", bufs=2, space="PSUM") as psM,
            tc.tile_pool(name="ps_st", bufs=1, space="PSUM") as ps_st,
            tc.tile_pool(name="dram", bufs=1, space="DRAM") as dp,
        ):
            # ---------------- persistent tensors ----------------
            xstage = pp.tile([P, NB, D], f32)         # raw x, block-major
            xpT = pp.tile([P, 4, N + 2 * P], f16)     # Xn^T hi/lo fp16 pair
            GF = pp.tile([P, NB, CH], bf16)           # [x@WxT | x@(.5Wn)T]
            hr = pp.tile([P, NB, CH], bf16)           # relu'd un-normalized h
            identity = pp.tile([P, P], f32)
            identB = pp.tile([P, P], bf16)
            identH = pp.tile([P, P], f16)
            masks = pp.tile([8, 3, CAND], f16)
            indi = pp.tile([8, P], f16)
            ones1_r = pp.tile([1, P], f32r)
            diagS = pp.tile([P, CAND], bf16)
            wfg = pp.tile([P, 2, CH], f16)
            ssq = pp.tile([P, NB], f32)
            inv = pp.tile([P, NB], f32)
            norms = pp.tile([P, NB], f32)
            sAall = pp.tile([P, NB], f32)
            sBall = pp.tile([P, NB], f32)
            rinvall = pp.tile([P, NB], f32)
            rball = pp.tile([P, NB], bf16)
            rb2all = pp.tile([P, NB], bf16)
            gam4 = pp.tile([4, P], f32)
            bet4 = pp.tile([4, P], f32)
            muq = pp.tile([4, P], f32)
            e2q = pp.tile([4, P], f32)
            v4 = pp.tile([4, P], f32)
            s4 = pp.tile([4, P], f32)
            b4 = pp.tile([4, P], f32)
            srow = pp.tile([1, CH], f32r)
            brow = pp.tile([1, CH], f32r)
            sbc = pp.tile([P, CH], bf16)
            bbc = pp.tile([P, CH], f32)
            st_sb = pp.tile([33, CH], f32)
            zerosB = pp.tile([P, DOUT], bf16)
            eps_t = pp.tile([4, 1], f32)
            if not nobias:
                bxn_row = pp.tile([1, CH], f32r)
                invT2 = pp.tile([1, NB, P], f32r)
                invT = pp.tile([NB, P], f32)

            make_identity(nc, identity)
            nc.vector.tensor_copy(identB, identity)
            nc.vector.tensor_copy(identH, identity)
            nc.gpsimd.memset(eps_t, EPS_BN)
            ones_rowf = pp.tile([1, P], f32)
            nc.gpsimd.memset(ones_rowf, 1.0)
            nc.vector.tensor_copy(ones1_r, ones_rowf)
            zscr = wp.tile([P, P], f32, tag="zpad")
            nc.gpsimd.memset(zscr, 0.0)
            for c in range(4):
                nc.vector.tensor_copy(xpT[:, c, 0:P], zscr)
                nc.vector.tensor_copy(xpT[:, c, N + P:N + 2 * P], zscr)
            nc.gpsimd.memset(diagS, 0.0)
            nc.vector.tensor_copy(diagS[:, P:2 * P], identB)
            nc.gpsimd.memset(zerosB, 0.0)

            nc.sync.dma_start(masks, wm_in[:, :, :])
            nc.sync.dma_start(indi, ind_in[:, :])
            for c in range(2):
                nc.sync.dma_start(wfg[:, c, :], wfg_in[P * c:P * (c + 1), :])
            nc.sync.dma_start(
                gam4, gamma_in[:, :].rearrange("a (q c) -> (a q) c", c=P))
            nc.sync.dma_start(
                bet4, beta_in[:, :].rearrange("a (q c) -> (a q) c", c=P))
            if not nobias:
                nc.sync.dma_start(bxn_row, bxn_in[:, :].bitcast(f32r))

            # x in 8 chunked DMAs of 4 blocks each
            for j in range(8):
                nc.sync.dma_start(
                    xstage[:, 4 * j:4 * (j + 1), :],
                    x_in[512 * j:512 * (j + 1), :]
                    .rearrange("(z p) d -> p z d", p=P))

            # ---------------- setup A: row norms ----------------
            for z in range(NB):
                xsq = wp.tile([P, D], bf16, tag="xsq")
                nc.vector.scalar_tensor_tensor(
                    out=xsq, in0=xstage[:, z, :], scalar=1.0,
                    in1=xstage[:, z, :],
                    op0=OP.mult, op1=OP.mult, accum_out=ssq[:, z:z + 1])
                if z % 4 == 3:
                    sl = slice(z - 3, z + 1)
                    nc.scalar.activation(out=norms[:, sl], in_=ssq[:, sl],
                                         func=AF.Sqrt)
                    nc.vector.reciprocal(out=inv[:, sl], in_=norms[:, sl])
            if not nobias:
                trv_ps = ps_st.tile([NB, P], f32, tag="sth")
                nc.tensor.transpose(trv_ps, inv[:, 0:NB], identity)
                nc.vector.tensor_copy(invT, trv_ps)
                invT_d = dp.tile([NB, P], f32)
                nc.sync.dma_start(invT_d, invT)
                nc.sync.dma_start(
                    invT2,
                    invT_d[:, :].rearrange("a b -> (a b)")[None, :].bitcast(f32r))

            st_h = ps_st.tile([33, CH], f32, tag="sth")

            # ---------------- staged setup + main loop ----------------
            def stage_B(z):
                # xh = fp16(x*inv); xl = fp16(x*inv - xh)
                xh = wp.tile([P, D], f16, tag="xh")
                nc.scalar.activation(out=xh, in_=xstage[:, z, :],
                                     func=AF.Copy, scale=inv[:, z:z + 1])
                xl = wp.tile([P, D], f16, tag="xl")
                nc.vector.scalar_tensor_tensor(
                    out=xl, in0=xstage[:, z, :], scalar=inv[:, z:z + 1],
                    in1=xh, op0=OP.mult, op1=OP.subtract)
                tr_ps = psF.tile([P, 4, P], f16, tag="ft")
                for c in range(2):
                    nc.tensor.transpose(tr_ps[:, c, :], xh[:, P * c:P * (c + 1)],
                                        identH)
                    nc.tensor.transpose(tr_ps[:, 2 + c, :],
                                        xl[:, P * c:P * (c + 1)], identH)
                ccol = P * (z + 1)
                nc.scalar.activation(out=xpT[:, :, ccol:ccol + P],
                                     in_=tr_ps[:, :, :], func=AF.Copy)
                # [G|F](z) = x @ [WxT | 0.5 WnT] (+ bias, inv-folded)
                fg_ps = psF.tile([P, CH], f32, tag="ft")
                for c in range(2):
                    nc.tensor.matmul(fg_ps, xpT[:, c, ccol:ccol + P],
                                     wfg[:, c, :], start=(c == 0),
                                     stop=(c == 1 and nobias))
                if not nobias:
                    nc.tensor.matmul(fg_ps, invT2[:, z, :], bxn_row,
                                     start=False, stop=True)
                nc.scalar.activation(out=GF[:, z, :], in_=fg_ps,
                                     func=AF.Copy, scale=norms[:, z:z + 1])

            def stage_M1(z):
                cstart = P * (z + 1)
                sim_ps = psS.tile([P, CAND], f32, tag="sim")
                first = True
                for c in range(2):
                    for (qs, cs) in ((0, 0), (0, 2), (2, 0)):
                        nc.tensor.matmul(sim_ps,
                                         xpT[:, qs + c, cstart:cstart + P],
                                         xpT[:, cs + c, P * z:P * z + CAND],
                                         start=first, stop=False)
                        first = False
                v = 0 if z == 0 else (2 if z == NB - 1 else 1)
                nc.tensor.matmul(sim_ps, indi, masks[:, v, :],
                                 start=False, stop=True)
                top8 = wp.tile([P, 8], f32, tag="top8")
                nc.vector.max(out=top8, in_=sim_ps)
                return sim_ps, top8

            def stage_M2(z, sim_ps, top8):
                Cm = wp.tile([P, CAND], bf16, tag="cm")
                nc.vector.scalar_tensor_tensor(out=Cm, in0=sim_ps,
                                               scalar=top8[:, 2:3], in1=diagS,
                                               op0=OP.is_ge, op1=OP.subtract)
                ct_ps = psM.tile([P, 3, P], bf16, tag="m")
                for k in range(3):
                    nc.tensor.transpose(ct_ps[:, k, :], Cm[:, P * k:P * (k + 1)],
                                        identB)
                ct_sb = wp.tile([P, 3, P], bf16, tag="ct_sb")
                nc.vector.tensor_copy(ct_sb, ct_ps)
                return ct_sb

            def stage_M3(z, ct_sb):
                h2_ps = psM.tile([P, DOUT], f32, tag="m")
                for k in range(3):
                    zk = min(max(z - 1 + k, 0), NB - 1)
                    nc.tensor.matmul(h2_ps, ct_sb[:, k, :],
                                     GF[:, zk, DOUT:CH],
                                     start=(k == 0), stop=(k == 2))
                # un-normalized relu halves + row sum-squares
                nc.scalar.activation(out=hr[:, z, DOUT:CH], in_=h2_ps,
                                     func=AF.Relu)
                scrB = wp.tile([P, DOUT], bf16, tag="scrB")
                nc.scalar.activation(out=scrB, in_=h2_ps, func=AF.Square,
                                     accum_out=sBall[:, z:z + 1])
                scrA = wp.tile([P, DOUT], bf16, tag="scrA")
                nc.vector.scalar_tensor_tensor(
                    out=scrA, in0=GF[:, z, 0:DOUT], scalar=1.0,
                    in1=GF[:, z, 0:DOUT], op0=OP.mult, op1=OP.mult,
                    accum_out=sAall[:, z:z + 1])
                nc.vector.tensor_scalar_max(hr[:, z, 0:DOUT],
                                            GF[:, z, 0:DOUT], 0.0)

            def stage_grp(z3):
                # batched rinv for blocks z3..z3+3
                sl = slice(z3, z3 + 4)
                tot4 = wp.tile([P, 4], f32, tag="tot4")
                nc.vector.tensor_add(tot4, sAall[:, sl], sBall[:, sl])
                hno4 = wp.tile([P, 4], f32, tag="hno4")
                nc.scalar.activation(out=hno4, in_=tot4, func=AF.Sqrt)
                nc.vector.reciprocal(out=rinvall[:, sl], in_=hno4)
                nc.vector.tensor_copy(rball[:, sl], rinvall[:, sl])
                nc.vector.tensor_mul(rb2all[:, sl], rinvall[:, sl],
                                     rinvall[:, sl])

            def stage_M4(z):
                hq = wp.tile([P, CH], bf16, tag="hq")
                nc.gpsimd.tensor_mul(hq, hr[:, z, :], hr[:, z, :])
                nc.tensor.matmul(st_h[0:1, :], rball[:, z:z + 1], hr[:, z, :],
                                 start=(z == 0), stop=(z == NB - 1))
                nc.tensor.matmul(st_h[32:33, :], rb2all[:, z:z + 1], hq,
                                 start=(z == 0), stop=(z == NB - 1))

            LAG4 = 8
            pend = {}
            for i in range(NB + LAG4):
                if i < NB:
                    stage_B(i)
                z4 = i - LAG4
                if 0 <= z4 < NB:
                    if z4 % 4 == 0:
                        stage_grp(z4)
                    stage_M4(z4)
                z3 = i - 3
                if 0 <= z3 < NB:
                    stage_M3(z3, pend.pop(("ct", z3)))
                z2 = i - 2
                if 0 <= z2 < NB:
                    sim_ps, top8 = pend.pop(("s", z2))
                    pend[("ct", z2)] = stage_M2(z2, sim_ps, top8)
                z1 = i - 1
                if 0 <= z1 < NB:
                    pend[("s", z1)] = stage_M1(z1)

            # ---------------- BN stats all-reduce ----------------
            st_in_d = dp.tile([2, CH], f32)
            st_out_d = dp.tile([2, CH], f32)
            nc.vector.tensor_copy(st_sb[0:1, :], st_h[0:1, :])
            nc.vector.tensor_copy(st_sb[32:33, :], st_h[32:33, :])
            nc.sync.dma_start(st_in_d[0:1, :], st_sb[0:1, :])
            nc.sync.dma_start(st_in_d[1:2, :], st_sb[32:33, :])
            if single:
                nc.sync.dma_start(st_out_d, st_in_d[:, :])
            else:
                nc.gpsimd.collective_compute(
                    "AllReduce", mybir.AluOpType.add,
                    replica_groups=[list(range(NCORES))],
                    ins=[st_in_d[:].opt()],
                    outs=[st_out_d[:].opt()],
                )
            sc = 1.0 / float(B * N)
            nc.sync.dma_start(
                muq, st_out_d[0:1, :].rearrange("a (q c) -> (a q) c", c=P))
            nc.sync.dma_start(
                e2q, st_out_d[1:2, :].rearrange("a (q c) -> (a q) c", c=P))
            nc.vector.tensor_scalar_mul(muq, muq, sc)          # mu
            nc.vector.tensor_scalar_mul(e2q, e2q, sc)          # E[h^2]
            nc.vector.tensor_mul(v4, muq, muq)                 # mu^2
            nc.vector.tensor_sub(v4, e2q, v4)                  # var
            nc.scalar.activation(out=v4, in_=v4, func=AF.Sqrt,
                                 bias=eps_t)
            nc.vector.reciprocal(out=v4, in_=v4)               # rstd
            nc.vector.tensor_mul(s4, v4, gam4)                 # s = gamma*rstd
            nc.vector.tensor_mul(b4, muq, s4)
            nc.vector.tensor_sub(b4, bet4, b4)                 # b = beta - mu*s
            # bounce s/b through DRAM to get [1, CH] row layout
            sb_d = dp.tile([2, 4, P], f32)
            nc.sync.dma_start(sb_d[0], s4)
            nc.sync.dma_start(sb_d[1], b4)
            nc.sync.dma_start(
                srow, sb_d[0].rearrange("q c -> (q c)")[None, :].bitcast(f32r))
            nc.sync.dma_start(
                brow, sb_d[1].rearrange("q c -> (q c)")[None, :].bitcast(f32r))
            bc_ps = psS.tile([P, CH], f32, tag="sim")
            nc.tensor.matmul(bc_ps, ones1_r, srow, start=True, stop=True)
            nc.vector.tensor_copy(sbc, bc_ps)
            bc_ps2 = psS.tile([P, CH], f32, tag="sim")
            nc.tensor.matmul(bc_ps2, ones1_r, brow, start=True, stop=True)
            nc.vector.tensor_copy(bbc, bc_ps2)

            # ---------------- BN apply + writeback ----------------
            # out = (hr * rinv_row) * s_ch + b_ch
            for z in range(NB):
                tmul = wp.tile([P, CH], bf16, tag="tmul")
                nc.vector.scalar_tensor_tensor(
                    out=tmul, in0=hr[:, z, :], scalar=rinvall[:, z:z + 1],
                    in1=sbc, op0=OP.mult, op1=OP.mult)
                out_t = wp.tile([P, CH], f32, tag="out_t")
                if z % 3 == 2:
                    nc.gpsimd.tensor_add(out_t, tmul, bbc)
                else:
                    nc.vector.tensor_add(out_t, tmul, bbc)
                nc.sync.dma_start(out_ext[P * z:P * (z + 1), :], out_t)

    return _finish(nc)


def _finish(nc):
    nc.finalize()
    return nc


def _get_nc(nobias=True):
    with _lock:
        key = ("nc", nobias)
        if key not in _cache:
            _cache[key] = _build(nobias=nobias)
        return _cache[key]


def _host_params():
    wm = np.full((8, 3, CAND), -60000.0, dtype=np.float16)
    for g in range(8):
        lo, hi = 16 * g, 16 * g + 272
        wm[g, 1, lo:hi] = 0.0                      # interior
        wm[g, 0, max(lo, 128):hi] = 0.0            # z == 0 (left edge)
        wm[g, 2, lo:min(hi, 256)] = 0.0            # z == NB-1 (right edge)
    ind = np.zeros((8, P), dtype=np.float16)
    for g in range(8):
        ind[g, 16 * g:16 * g + 16] = 1.0
    return wm, ind


def _run(inputs, trace=False, trace_kwargs=None):
    from concourse.bass_utils import run_bass_kernel_spmd

    x = np.ascontiguousarray(np.asarray(inputs["x"], dtype=np.float32))
    Wx_w = np.asarray(inputs["Wx_w"], dtype=np.float32)
    Wx_b = np.asarray(inputs["Wx_b"], dtype=np.float32)
    Wn_w = np.asarray(inputs["Wn_w"], dtype=np.float32)
    Wn_b = np.asarray(inputs["Wn_b"], dtype=np.float32)
    gamma = np.asarray(inputs["gamma"], dtype=np.float32)
    beta = np.asarray(inputs["beta"], dtype=np.float32)
    assert x.shape == (B, N, D), x.shape
    assert int(inputs["p"]) == 16 and int(inputs["t"]) == 8

    nobias = bool(np.all(Wx_b == 0.0) and np.all(Wn_b == 0.0))
    wfg = np.concatenate([Wx_w.T, (0.5 * Wn_w).T], axis=1).astype(np.float16)
    wm, ind = _host_params()
    shared = {
        "wfg": np.ascontiguousarray(wfg), "winmask": wm, "indi": ind,
        "gamma": gamma.reshape(1, CH), "beta": beta.reshape(1, CH),
    }
    if not nobias:
        shared["bxn"] = np.concatenate(
            [Wx_b, 0.5 * Wn_b]).reshape(1, CH).astype(np.float32)
    in_maps = [{"xb": np.ascontiguousarray(x[c]), **shared} for c in range(NCORES)]

    nc = _get_nc(nobias=nobias)
    kw = {}
    if trace:
        kw = dict(trace=True, trace_kwargs=trace_kwargs or {})
    res = run_bass_kernel_spmd(nc, in_maps, core_ids=list(range(NCORES)), **kw)
    out = np.stack([res.results[c]["out"] for c in range(NCORES)], axis=0)
    return out.astype(np.float32), res


def kernel(**inputs):
    out, _ = _run(inputs)
    return out


# revision 34
# speedup vs baseline: 1.4888x; 1.0194x over previous
"""BatchedGraphSAGEDynamicRangeMean kernel for 8 Trainium2 NeuronCores.

Sharding: data-parallel over batch b -- core c computes graph c entirely
(N=4096 nodes, D=256), BN statistics all-reduced across the 8 cores.

Per-core pipeline (32 row-blocks of 128 nodes), software-pipelined with
staged emission so every engine always has ready work:
  setup B(z): xh/xl = fp16 hi/lo split of xn = x*rsqrt(ssq) (exact fp32
    pair), 4 PE transposes -> XnT hi/lo; [G|F] = x @ [Wx^T | 0.5*Wn^T]
    per block (norm un-scale folded into the PSUM->SBUF copy, bf16).
  M1(z): banded cosine sims = 6 fp16 matmuls (hh+hl+lh, fp32-exact) +
    window-validity mask folded in as a rank-8 fp16 matmul accumulated
    into the same PSUM group; hardware max8 -> v3 = 3rd-largest.
  M2(z): C = (S >= v3) - selfdiag (bf16, exact); 3 PE transposes.
  M3(z): h2 = C^T-chunks @ F rows (neighbour mean + Wn fused); store
    UN-normalized relu halves hr = [relu(G), relu(h2)] (bf16) -- the row
    1/l2norm scale is deferred, so nothing on this path waits for it.
  M4(z): row sum-squares of G / h2 (fused square+row-reduce); hq = hr^2;
    BN partial sums via matmuls whose lhsT column carries rinv / rinv^2.
  Every 4 blocks: batched scale chain rinv = 1/sqrt(sumsq) on [P,4].
  AllReduce(2x512) -> channel scale/bias; apply = one fused
  (hr*rinv)*sbc multiply (per-row x per-channel) + bias add + DMA.
"""

import threading
import numpy as np

B, N, D, DOUT = 8, 4096, 256, 256
P = 128
NB = N // P            # 32 blocks
CAND = 3 * P           # 384 candidate columns per block
NCORES = 8
CH = 2 * DOUT          # 512 output channels
EPS_BN = 1e-5

_cache = {}
_lock = threading.Lock()


def _build(single=False, nobias=True):
    import concourse.bass as bass
    from concourse import bacc
    import concourse.mybir as mybir
    import concourse.tile as tile
    from concourse.masks import make_identity

    f32 = mybir.dt.float32
    f32r = mybir.dt.float32r
    bf16 = mybir.dt.bfloat16
    f16 = mybir.dt.float16
    AF = mybir.ActivationFunctionType
    OP = mybir.AluOpType

    nc = bacc.Bacc("TRN2", target_bir_lowering=False)
    x_in = nc.declare_dram_parameter("xb", [N, D], f32, isOutput=False)
    wfg_in = nc.declare_dram_parameter("wfg", [D, CH], f16, isOutput=False)
    gamma_in = nc.declare_dram_parameter("gamma", [1, CH], f32, isOutput=False)
    beta_in = nc.declare_dram_parameter("beta", [1, CH], f32, isOutput=False)
    wm_in = nc.declare_dram_parameter("winmask", [8, 3, CAND], f16, isOutput=False)
    ind_in = nc.declare_dram_parameter("indi", [8, P], f16, isOutput=False)
    eq_in = nc.declare_dram_parameter("eq4", [4, 4 * P], f32, isOutput=False)
    if not nobias:
        bxn_in = nc.declare_dram_parameter("bxn", [1, CH], f32, isOutput=False)
    out_ext = nc.declare_dram_parameter("out", [N, CH], f32, isOutput=True)

    with tile.TileContext(nc) as tc:
        with (
            tc.tile_pool(name="persist", bufs=1) as pp,
            tc.tile_pool(name="work", bufs=3) as wp,
            tc.tile_pool(name="psS", bufs=2, space="PSUM") as psS,
            tc.tile_pool(name="psF", bufs=2, space="PSUM") as psF,
            tc.tile_pool(name="psM", bufs=2, space="PSUM") as psM,
            tc.tile_pool(name="ps_st", bufs=1, space="PSUM") as ps_st,
            tc.tile_pool(name="dram", bufs=1, space="DRAM") as dp,
        ):
            # ---------------- persistent tensors ----------------
            xstage = pp.tile([P, NB, D], f32)         # raw x, block-major
            xpT = pp.tile([P, 4, N + 2 * P], f16)     # Xn^T hi/lo fp16 pair
            GF = pp.tile([P, NB, CH], bf16)           # [x@WxT | x@(.5Wn)T]
            hr = pp.tile([P, NB, CH], bf16)           # relu'd un-normalized h
            identity = pp.tile([P, P], f32)
            identB = pp.tile([P, P], bf16)
            identH = pp.tile([P, P], f16)
            masks = pp.tile([8, 3, CAND], f16)
            indi = pp.tile([8, P], f16)
            diagS = pp.tile([P, CAND], bf16)
            wfg = pp.tile([P, 2, CH], f16)
            ssq = pp.tile([P, NB], f32)
            inv = pp.tile([P, NB], f32)
            norms = pp.tile([P, NB], f32)
            sAall = pp.tile([P, NB], f32)
            sBall = pp.tile([P, NB], f32)
            rinvall = pp.tile([P, NB], f32)
            rball = pp.tile([P, NB], bf16)
            rb2all = pp.tile([P, NB], bf16)
            gam4 = pp.tile([4, P], f32)
            bet4 = pp.tile([4, P], f32)
            muq = pp.tile([4, P], f32)
            e2q = pp.tile([4, P], f32)
            v4 = pp.tile([4, P], f32)
            s4 = pp.tile([4, P], f32)
            b4 = pp.tile([4, P], f32)
            eq4f = pp.tile([4, 4 * P], f32)
            eq4b = pp.tile([4, 4 * P], bf16)
            s4b = pp.tile([4, P], bf16)
            sbc = pp.tile([P, CH], bf16)
            bbc = pp.tile([P, CH], f32)
            st_sb = pp.tile([33, CH], f32)
            st_sb2 = pp.tile([33, CH], f32)
            muqB = pp.tile([4, P], f32)
            e2qB = pp.tile([4, P], f32)
            zerosB = pp.tile([P, DOUT], bf16)
            eps_t = pp.tile([4, 1], f32)
            if not nobias:
                bxn_row = pp.tile([1, CH], f32r)
                invT2 = pp.tile([1, NB, P], f32r)
                invT = pp.tile([NB, P], f32)

            make_identity(nc, identity)
            nc.vector.tensor_copy(identB, identity)
            nc.vector.tensor_copy(identH, identity)
            nc.gpsimd.memset(eps_t, EPS_BN)
            zscr = wp.tile([P, P], f32, tag="zpad")
            nc.gpsimd.memset(zscr, 0.0)
            for c in range(4):
                nc.vector.tensor_copy(xpT[:, c, 0:P], zscr)
                nc.vector.tensor_copy(xpT[:, c, N + P:N + 2 * P], zscr)
            nc.gpsimd.memset(diagS, 0.0)
            nc.vector.tensor_copy(diagS[:, P:2 * P], identB)
            nc.gpsimd.memset(zerosB, 0.0)

            nc.sync.dma_start(masks, wm_in[:, :, :])
            nc.sync.dma_start(indi, ind_in[:, :])
            nc.sync.dma_start(eq4f, eq_in[:, :])
            nc.vector.tensor_copy(eq4b, eq4f)
            for c in range(2):
                nc.sync.dma_start(wfg[:, c, :], wfg_in[P * c:P * (c + 1), :])
            nc.sync.dma_start(
                gam4, gamma_in[:, :].rearrange("a (q c) -> (a q) c", c=P))
            nc.sync.dma_start(
                bet4, beta_in[:, :].rearrange("a (q c) -> (a q) c", c=P))
            if not nobias:
                nc.sync.dma_start(bxn_row, bxn_in[:, :].bitcast(f32r))

            # x in 8 chunked DMAs of 4 blocks each
            for j in range(8):
                nc.sync.dma_start(
                    xstage[:, 4 * j:4 * (j + 1), :],
                    x_in[512 * j:512 * (j + 1), :]
                    .rearrange("(z p) d -> p z d", p=P))

            # ---------------- setup A: row norms (interleaved) ----------------
            def stage_A(z):
                xsq = wp.tile([P, D], bf16, tag="xsq")
                nc.vector.scalar_tensor_tensor(
                    out=xsq, in0=xstage[:, z, :], scalar=1.0,
                    in1=xstage[:, z, :],
                    op0=OP.mult, op1=OP.mult, accum_out=ssq[:, z:z + 1])
                if z % 4 == 3:
                    sl = slice(z - 3, z + 1)
                    nc.scalar.activation(out=norms[:, sl], in_=ssq[:, sl],
                                         func=AF.Sqrt)
                    nc.vector.reciprocal(out=inv[:, sl], in_=norms[:, sl])

            for z in range(8 if nobias else NB):
                stage_A(z)
            if not nobias:
                trv_ps = ps_st.tile([NB, P], f32, tag="sth")
                nc.tensor.transpose(trv_ps, inv[:, 0:NB], identity)
                nc.vector.tensor_copy(invT, trv_ps)
                invT_d = dp.tile([NB, P], f32)
                nc.sync.dma_start(invT_d, invT)
                nc.sync.dma_start(
                    invT2,
                    invT_d[:, :].rearrange("a b -> (a b)")[None, :].bitcast(f32r))

            st_h = ps_st.tile([33, CH], f32, tag="sth")
            st_hB = ps_st.tile([33, CH], f32, tag="sthB")
            STSPLIT = 28
            stA_in_d = dp.tile([2, CH], f32)
            stA_out_d = dp.tile([2, CH], f32)
            stB_in_d = dp.tile([2, CH], f32)
            stB_out_d = dp.tile([2, CH], f32)

            # ---------------- staged setup + main loop ----------------
            def stage_B(z):
                # xh = fp16(x*inv); xl = fp16(x*inv - xh)
                xh = wp.tile([P, D], f16, tag="xh")
                nc.scalar.activation(out=xh, in_=xstage[:, z, :],
                                     func=AF.Copy, scale=inv[:, z:z + 1])
                xl = wp.tile([P, D], f16, tag="xl")
                nc.vector.scalar_tensor_tensor(
                    out=xl, in0=xstage[:, z, :], scalar=inv[:, z:z + 1],
                    in1=xh, op0=OP.mult, op1=OP.subtract)
                tr_ps = psF.tile([P, 4, P], f16, tag="ft")
                for c in range(2):
                    nc.tensor.transpose(tr_ps[:, c, :], xh[:, P * c:P * (c + 1)],
                                        identH)
                    nc.tensor.transpose(tr_ps[:, 2 + c, :],
                                        xl[:, P * c:P * (c + 1)], identH)
                ccol = P * (z + 1)
                nc.scalar.activation(out=xpT[:, :, ccol:ccol + P],
                                     in_=tr_ps[:, :, :], func=AF.Copy)
                # [G|F](z) = x @ [WxT | 0.5 WnT] (+ bias, inv-folded)
                fg_ps = psF.tile([P, CH], f32, tag="ft")
                for c in range(2):
                    nc.tensor.matmul(fg_ps, xpT[:, c, ccol:ccol + P],
                                     wfg[:, c, :], start=(c == 0),
                                     stop=(c == 1 and nobias))
                if not nobias:
                    nc.tensor.matmul(fg_ps, invT2[:, z, :], bxn_row,
                                     start=False, stop=True)
                nc.scalar.activation(out=GF[:, z, :], in_=fg_ps,
                                     func=AF.Copy, scale=norms[:, z:z + 1])

            def stage_M1(z):
                cstart = P * (z + 1)
                sim_ps = psS.tile([P, CAND], f32, tag="sim")
                first = True
                for c in range(2):
                    for (qs, cs) in ((0, 0), (0, 2), (2, 0)):
                        nc.tensor.matmul(sim_ps,
                                         xpT[:, qs + c, cstart:cstart + P],
                                         xpT[:, cs + c, P * z:P * z + CAND],
                                         start=first, stop=False)
                        first = False
                v = 0 if z == 0 else (2 if z == NB - 1 else 1)
                nc.tensor.matmul(sim_ps, indi, masks[:, v, :],
                                 start=False, stop=True)
                top8 = wp.tile([P, 8], f32, tag="top8")
                nc.vector.max(out=top8, in_=sim_ps)
                return sim_ps, top8

            def stage_M2(z, sim_ps, top8):
                Cm = wp.tile([P, CAND], bf16, tag="cm")
                nc.vector.scalar_tensor_tensor(out=Cm, in0=sim_ps,
                                               scalar=top8[:, 2:3], in1=diagS,
                                               op0=OP.is_ge, op1=OP.subtract)
                ct_ps = psM.tile([P, 3, P], bf16, tag="m")
                for k in range(3):
                    nc.tensor.transpose(ct_ps[:, k, :], Cm[:, P * k:P * (k + 1)],
                                        identB)
                ct_sb = wp.tile([P, 3, P], bf16, tag="ct_sb")
                nc.vector.tensor_copy(ct_sb, ct_ps)
                return ct_sb

            def stage_M3(z, ct_sb):
                h2_ps = psM.tile([P, DOUT], f32, tag="m")
                for k in range(3):
                    zk = min(max(z - 1 + k, 0), NB - 1)
                    nc.tensor.matmul(h2_ps, ct_sb[:, k, :],
                                     GF[:, zk, DOUT:CH],
                                     start=(k == 0), stop=(k == 2))
                # un-normalized relu halves + row sum-squares
                nc.scalar.activation(out=hr[:, z, DOUT:CH], in_=h2_ps,
                                     func=AF.Relu)
                scrB = wp.tile([P, DOUT], bf16, tag="scrB")
                nc.scalar.activation(out=scrB, in_=h2_ps, func=AF.Square,
                                     accum_out=sBall[:, z:z + 1])
                scrA = wp.tile([P, DOUT], bf16, tag="scrA")
                nc.vector.scalar_tensor_tensor(
                    out=scrA, in0=GF[:, z, 0:DOUT], scalar=1.0,
                    in1=GF[:, z, 0:DOUT], op0=OP.mult, op1=OP.mult,
                    accum_out=sAall[:, z:z + 1])
                nc.vector.tensor_scalar_max(hr[:, z, 0:DOUT],
                                            GF[:, z, 0:DOUT], 0.0)

            def stage_grp(z3):
                # batched rinv for blocks z3..z3+3
                sl = slice(z3, z3 + 4)
                tot4 = wp.tile([P, 4], f32, tag="tot4")
                nc.vector.tensor_add(tot4, sAall[:, sl], sBall[:, sl])
                hno4 = wp.tile([P, 4], f32, tag="hno4")
                nc.scalar.activation(out=hno4, in_=tot4, func=AF.Sqrt)
                nc.vector.reciprocal(out=rinvall[:, sl], in_=hno4)
                nc.vector.tensor_copy(rball[:, sl], rinvall[:, sl])
                nc.vector.tensor_mul(rb2all[:, sl], rinvall[:, sl],
                                     rinvall[:, sl])

            def stage_M4(z):
                hq = wp.tile([P, CH], bf16, tag="hq")
                nc.gpsimd.tensor_mul(hq, hr[:, z, :], hr[:, z, :])
                st = st_h if z < STSPLIT else st_hB
                z0, z1 = (0, STSPLIT - 1) if z < STSPLIT else (STSPLIT, NB - 1)
                nc.tensor.matmul(st[0:1, :], rball[:, z:z + 1], hr[:, z, :],
                                 start=(z == z0), stop=(z == z1))
                nc.tensor.matmul(st[32:33, :], rb2all[:, z:z + 1], hq,
                                 start=(z == z0), stop=(z == z1))

            LAG4 = 8
            pend = {}
            for i in range(NB + LAG4):
                if nobias and 8 + i < NB:
                    stage_A(8 + i)
                if i < NB:
                    stage_B(i)
                z4 = i - LAG4
                if 0 <= z4 < NB:
                    if z4 % 4 == 0:
                        stage_grp(z4)
                    stage_M4(z4)
                z3 = i - 3
                if 0 <= z3 < NB:
                    stage_M3(z3, pend.pop(("ct", z3)))
                z2 = i - 2
                if 0 <= z2 < NB:
                    sim_ps, top8 = pend.pop(("s", z2))
                    pend[("ct", z2)] = stage_M2(z2, sim_ps, top8)
                z1 = i - 1
                if 0 <= z1 < NB:
                    pend[("s", z1)] = stage_M1(z1)
                if i == STSPLIT + LAG4:
                    # early AllReduce over blocks 0..STSPLIT-1, overlapped
                    nc.vector.tensor_copy(st_sb[0:1, :], st_h[0:1, :])
                    nc.vector.tensor_copy(st_sb[32:33, :], st_h[32:33, :])
                    nc.sync.dma_start(stA_in_d[0:1, :], st_sb[0:1, :])
                    nc.sync.dma_start(stA_in_d[1:2, :], st_sb[32:33, :])
                    if single:
                        nc.sync.dma_start(stA_out_d, stA_in_d[:, :])
                    else:
                        nc.gpsimd.collective_compute(
                            "AllReduce", mybir.AluOpType.add,
                            replica_groups=[list(range(NCORES))],
                            ins=[stA_in_d[:].opt()],
                            outs=[stA_out_d[:].opt()],
                        )
                    nc.sync.dma_start(
                        muq,
                        stA_out_d[0:1, :].rearrange("a (q c) -> (a q) c", c=P))
                    nc.sync.dma_start(
                        e2q,
                        stA_out_d[1:2, :].rearrange("a (q c) -> (a q) c", c=P))

            # ---------------- tail stats all-reduce (blocks 28..31) ----------
            nc.vector.tensor_copy(st_sb2[0:1, :], st_hB[0:1, :])
            nc.vector.tensor_copy(st_sb2[32:33, :], st_hB[32:33, :])
            nc.sync.dma_start(stB_in_d[0:1, :], st_sb2[0:1, :])
            nc.sync.dma_start(stB_in_d[1:2, :], st_sb2[32:33, :])
            if single:
                nc.sync.dma_start(stB_out_d, stB_in_d[:, :])
            else:
                nc.gpsimd.collective_compute(
                    "AllReduce", mybir.AluOpType.add,
                    replica_groups=[list(range(NCORES))],
                    ins=[stB_in_d[:].opt()],
                    outs=[stB_out_d[:].opt()],
                )
            sc = 1.0 / float(B * N)
            nc.sync.dma_start(
                muqB, stB_out_d[0:1, :].rearrange("a (q c) -> (a q) c", c=P))
            nc.sync.dma_start(
                e2qB, stB_out_d[1:2, :].rearrange("a (q c) -> (a q) c", c=P))
            nc.vector.tensor_add(muq, muq, muqB)
            nc.vector.tensor_add(e2q, e2q, e2qB)
            nc.vector.tensor_scalar_mul(muq, muq, sc)          # mu
            nc.vector.tensor_scalar_mul(e2q, e2q, sc)          # E[h^2]
            nc.vector.tensor_mul(v4, muq, muq)                 # mu^2
            nc.vector.tensor_sub(v4, e2q, v4)                  # var
            nc.scalar.activation(out=v4, in_=v4, func=AF.Sqrt,
                                 bias=eps_t)
            nc.vector.reciprocal(out=v4, in_=v4)               # rstd
            nc.vector.tensor_mul(s4, v4, gam4)                 # s = gamma*rstd
            nc.vector.tensor_mul(b4, muq, s4)
            nc.vector.tensor_sub(b4, bet4, b4)                 # b = beta - mu*s
            nc.vector.tensor_copy(s4b, s4)
            # broadcast via one-hot quarter matmuls (no DRAM bounce)
            bc_ps = psS.tile([P, CH], f32, tag="sim")
            bc_ps2 = psS.tile([P, CH], f32, tag="sim")
            for q in range(4):
                nc.tensor.matmul(bc_ps[:, P * q:P * (q + 1)],
                                 eq4b[:, P * q:P * (q + 1)], s4b,
                                 start=True, stop=True)
                nc.tensor.matmul(bc_ps2[:, P * q:P * (q + 1)],
                                 eq4f[:, P * q:P * (q + 1)], b4,
                                 start=True, stop=True)
            nc.vector.tensor_copy(sbc, bc_ps)
            nc.vector.tensor_copy(bbc, bc_ps2)

            # ---------------- BN apply + writeback ----------------
            # out = ((hr * s_ch) * rinv_row) + b_ch
            for z in range(NB):
                if z % 2 == 0:
                    tmul = wp.tile([P, CH], bf16, tag="tmul")
                    nc.vector.tensor_mul(tmul, hr[:, z, :], sbc)
                    out_t = wp.tile([P, CH], f32, tag="out_t")
                    nc.vector.scalar_tensor_tensor(
                        out=out_t, in0=tmul, scalar=rinvall[:, z:z + 1],
                        in1=bbc, op0=OP.mult, op1=OP.add)
                else:
                    t1 = wp.tile([P, CH], bf16, tag="t1")
                    nc.scalar.activation(out=t1, in_=hr[:, z, :],
                                         func=AF.Copy,
                                         scale=rinvall[:, z:z + 1])
                    t2 = wp.tile([P, CH], bf16, tag="t2")
                    nc.vector.tensor_mul(t2, t1, sbc)
                    out_t = wp.tile([P, CH], f32, tag="out_t")
                    nc.gpsimd.tensor_add(out_t, t2, bbc)
                nc.sync.dma_start(out_ext[P * z:P * (z + 1), :], out_t)

    return _finish(nc)


def _finish(nc):
    nc.finalize()
    return nc


def _get_nc(nobias=True):
    with _lock:
        key = ("nc", nobias)
        if key not in _cache:
            _cache[key] = _build(nobias=nobias)
        return _cache[key]


def _host_params():
    wm = np.full((8, 3, CAND), -60000.0, dtype=np.float16)
    for g in range(8):
        lo, hi = 16 * g, 16 * g + 272
        wm[g, 1, lo:hi] = 0.0                      # interior
        wm[g, 0, max(lo, 128):hi] = 0.0            # z == 0 (left edge)
        wm[g, 2, lo:min(hi, 256)] = 0.0            # z == NB-1 (right edge)
    ind = np.zeros((8, P), dtype=np.float16)
    for g in range(8):
        ind[g, 16 * g:16 * g + 16] = 1.0
    eq = np.zeros((4, 4 * P), dtype=np.float32)
    for q in range(4):
        eq[q, P * q:P * (q + 1)] = 1.0
    return wm, ind, eq


def _run(inputs, trace=False, trace_kwargs=None):
    from concourse.bass_utils import run_bass_kernel_spmd

    x = np.ascontiguousarray(np.asarray(inputs["x"], dtype=np.float32))
    Wx_w = np.asarray(inputs["Wx_w"], dtype=np.float32)
    Wx_b = np.asarray(inputs["Wx_b"], dtype=np.float32)
    Wn_w = np.asarray(inputs["Wn_w"], dtype=np.float32)
    Wn_b = np.asarray(inputs["Wn_b"], dtype=np.float32)
    gamma = np.asarray(inputs["gamma"], dtype=np.float32)
    beta = np.asarray(inputs["beta"], dtype=np.float32)
    assert x.shape == (B, N, D), x.shape
    assert int(inputs["p"]) == 16 and int(inputs["t"]) == 8

    nobias = bool(np.all(Wx_b == 0.0) and np.all(Wn_b == 0.0))
    wfg = np.concatenate([Wx_w.T, (0.5 * Wn_w).T], axis=1).astype(np.float16)
    wm, ind, eq = _host_params()
    shared = {
        "wfg": np.ascontiguousarray(wfg), "winmask": wm, "indi": ind,
        "eq4": eq,
        "gamma": gamma.reshape(1, CH), "beta": beta.reshape(1, CH),
    }
    if not nobias:
        shared["bxn"] = np.concatenate(
            [Wx_b, 0.5 * Wn_b]).reshape(1, CH).astype(np.float32)
    in_maps = [{"xb": np.ascontiguousarray(x[c]), **shared} for c in range(NCORES)]

    nc = _get_nc(nobias=nobias)
    kw = {}
    if trace:
        kw = dict(trace=True, trace_kwargs=trace_kwargs or {})
    res = run_bass_kernel_spmd(nc, in_maps, core_ids=list(range(NCORES)), **kw)
    out = np.stack([res.results[c]["out"] for c in range(NCORES)], axis=0)
    return out.astype(np.float32), res


def kernel(**inputs):
    out, _ = _run(inputs)
    return out
